# revision 15
# baseline (speedup 1.0000x reference)
"""Trainium2 Bass kernel for nn_DecoderBlock (B=8, S=1024, D=256, H=4 heads
of full width 256, FF=1024).

Strategy: pure data parallelism — B=8 batch elements across 8 NeuronCores,
zero collectives. Per core, one full decoder block in "transposed" activation
layout (features on SBUF partitions, tokens on the free dim).

Perf tricks over the bf16 baseline:
- Scores use the host-precomputed per-head M = Wq^T Wk: scores = x^T M x,
  so only ONE projection g = M^T x per head is materialized (no separate
  q/k), halving projection matmuls and PSUM->SBUF copies.
- fp8e4 (TRN E4M3, max +-240) with perf_mode=DoubleRowSwInterleave for
  the attention core (scores, softmax denominator Z, att@V), the g/v
  projections and both FFN matmuls: 2 fp8 weights/cell => 0.5 PE cycles
  per output column, 256-deep contraction per instruction. (Plain
  DoubleRow hangs the exec unit on this silicon; SwInterleave with
  host/on-chip interleaved+column-reversed weights works.) On-chip
  produced weights (g, att@V's v) are written through positive-stride-2
  APs; their column reversal comes free by feeding the PRODUCING matmul
  block-reversed rhs data (xRD / wvRD). Scale management keeps every
  fp8 operand in the E4M3 normal range (x*8, M*1024, wv*64, ff1*64,
  ff2*32; exp() values are O(1) naturally).
- Causal masking via gpsimd (Pool) affine_select directly on the fp8 exp
  tiles; Pool also takes the LayerNorm applies + squares (it has no PSUM
  port, so PSUM-reading ops stay on ACT/DVE).
- LN mean stats matmuls read the f32 residual via a float32r bitcast
  (1 cycle/col at free>=256) so no bf16 twin of the residual is written.
- ACT activation-table discipline: attention uses only Exp; everything
  after uses {Square, Rsqrt, Copy, Relu} which share one table
  (reciprocal_sqrt_and_small) => a single table swap per kernel.

The attention_mask input is all ones per the problem spec (causal mask
only); if a mask with zeros ever shows up, we fall back to numpy.
"""

import numpy as np
import ml_dtypes

import concourse.bass as bass
import concourse.mybir as mybir
import concourse.tile as tile
from concourse import bacc
from concourse.bass_utils import run_bass_kernel_spmd

F32 = mybir.dt.float32
F32R = mybir.dt.float32r
BF16 = mybir.dt.bfloat16
FP8 = mybir.dt.float8e4
AF = mybir.ActivationFunctionType
ALU = mybir.AluOpType
DRS = mybir.MatmulPerfMode.DoubleRowSwInterleave

N_CORES = 8
B, S, D, H, E, HE, FF = 8, 1024, 256, 4, 256, 1024, 1024
SC = 512          # token (free-dim) chunk
NJ = S // SC      # 2 chunks of tokens
ND = D // 128     # 2 partition chunks of features
NF = FF // 128    # 8 partition chunks of ff features
NT = S // 128     # 8 partition chunks of tokens
LN_EPS = 1e-5
# scaling: x*8, M*4096 -> g psum = 32768*g, gD = 16*g (copy scale 1/2048)
# scores psum = gD*xD = 128 * (x^T M x) = 128*16*scores_ref -> exp scale:
EXP_SCALE = 1.0 / 2048.0
G_SCALE = 1.0 / 512.0    # g psum (8x*1024M = 8192g) -> gD (16*g)
V_SCALE = 1.0 / 32.0     # v psum (8x * 64wv = 512v) -> vhD (16*v)
FF2_SCALE = 1.0 / 2048.0  # ff2 psum (64h * 32w = 2048*out) -> out

_CACHE = {}


def _build():
    nc = bacc.Bacc("TRN2", target_bir_lowering=False, debug=False,
                   num_devices=N_CORES)

    # ---- DRAM parameters (per-core shard + replicated weights) ----
    xD_d = nc.dram_tensor("xD", [128, 2, S], FP8, kind="ExternalInput")
    xRD_d = nc.dram_tensor("xRD", [128, 2, S], FP8, kind="ExternalInput")
    xSW_d = nc.dram_tensor("xSW", [128, NT, 2 * 128], FP8,
                           kind="ExternalInput")
    MD_d = nc.dram_tensor("MDsw", [H, 128, 2, 2 * 128], FP8,
                          kind="ExternalInput")
    wvD_d = nc.dram_tensor("wvRD", [128, 2, HE], FP8, kind="ExternalInput")
    xT_d = nc.dram_tensor("xT", [ND, 128, S], F32, kind="ExternalInput")
    woT_d = nc.dram_tensor("woT", [NT, 128, D], BF16, kind="ExternalInput")
    ff1D_d = nc.dram_tensor("ff1sw", [128, NF, 2 * 128], FP8,
                            kind="ExternalInput")
    ff2D_d = nc.dram_tensor("ff2sw", [128, NF // 2, 2, 2 * 128], FP8,
                            kind="ExternalInput")
    wo_b_d = nc.dram_tensor("wo_b", [ND, 128, 1], F32, kind="ExternalInput")
    ff1b64_d = nc.dram_tensor("ff1b64", [NF, 128, 1], F32, kind="ExternalInput")
    ff2_b_d = nc.dram_tensor("ff2b_f", [ND, 128, 1], F32, kind="ExternalInput")
    ln1_g_d = nc.dram_tensor("ln1_g", [ND, 128, 1], F32, kind="ExternalInput")
    ln1_b_d = nc.dram_tensor("ln1_b", [ND, 128, 1], F32, kind="ExternalInput")
    ln2_g_d = nc.dram_tensor("ln2_g", [ND, 128, 1], F32, kind="ExternalInput")
    ln2_b_d = nc.dram_tensor("ln2_b", [ND, 128, 1], F32, kind="ExternalInput")
    out_d = nc.dram_tensor("out", [ND, 128, S], F32, kind="ExternalOutput")

    with tile.TileContext(nc) as tc:
        with tc.tile_pool(name="consts", bufs=1) as consts, \
             tc.tile_pool(name="acts", bufs=1) as acts, \
             tc.tile_pool(name="work", bufs=2) as work, \
             tc.tile_pool(name="psA", bufs=3, space="PSUM") as psA, \
             tc.tile_pool(name="psO", bufs=3, space="PSUM") as psO, \
             tc.tile_pool(name="psZ", bufs=2, space="PSUM") as psZ:

            def loadc(dram, shape, dt, name, src=None):
                t = consts.tile(shape, dt, tag=name, name=name)
                nc.sync.dma_start(out=t[:], in_=dram[:] if src is None else src)
                return t

            # DMA in first-use order so the PE can start ASAP
            xRD = loadc(xRD_d, [128, 2, S], FP8, "xRD")
            MDsw = [loadc(MD_d, [128, 2, 256], FP8, f"MDsw{h}", src=MD_d[h])
                    for h in range(H)]
            xSW = loadc(xSW_d, [128, NT, 256], FP8, "xSW")
            xD = loadc(xD_d, [128, 2, S], FP8, "xD")
            wvRD = loadc(wvD_d, [128, 2, HE], FP8, "wvRD")
            xT = [loadc(xT_d, [128, S], F32, f"xT{i}", src=xT_d[i])
                  for i in range(ND)]
            woT = [loadc(woT_d, [128, D], BF16, f"woT{i}", src=woT_d[i])
                   for i in range(NT)]
            ff1sw = loadc(ff1D_d, [128, NF, 256], FP8, "ff1sw")
            ff2sw = loadc(ff2D_d, [128, NF // 2, 2, 256], FP8, "ff2sw")

            def loadb(dram, n, name):
                return [loadc(dram, [128, 1], F32, f"{name}{i}", src=dram[i])
                        for i in range(n)]
            wo_b = loadb(wo_b_d, ND, "wo_b")
            ff1b64 = loadb(ff1b64_d, NF, "ff1b64")
            ff2b_f = loadb(ff2_b_d, ND, "ff2b_f")
            ln1_g = loadb(ln1_g_d, ND, "ln1_g")
            ln1_b = loadb(ln1_b_d, ND, "ln1_b")
            ln2_g = loadb(ln2_g_d, ND, "ln2_g")
            ln2_b = loadb(ln2_b_d, ND, "ln2_b")

            # constants
            onesSW = consts.tile([128, 256], FP8, tag="onesSW", name="onesSW")
            nc.vector.memset(onesSW[:], 16.0)  # matches the 16*v scale
            invd_bf = consts.tile([128, 128], BF16, tag="invdb", name="invdb")
            nc.vector.memset(invd_bf[:], 1.0 / D)
            eps_t = consts.tile([128, 1], F32, tag="eps", name="eps")
            nc.vector.memset(eps_t[:], LN_EPS)
            warm_bf = consts.tile([128, 128], BF16, tag="warmw", name="warmw")
            nc.vector.memset(warm_bf[:], 0.0)

            # PE warmup: dummy matmuls (no DMA dependency) keep the HAM
            # clock gate ramping while the input DMAs land.
            def warm_pe(n):
                for _ in range(n):
                    wp = psA.tile([128, SC], F32, tag="mm", name="warm")
                    nc.tensor.matmul(wp[:, :128], warm_bf[:], warm_bf[:],
                                     start=True, stop=True)

            warm_pe(22)

            # ---- attention ----
            # ONT [HE, S] normalized head outputs (bf16) for the wo matmul
            ont = [acts.tile([128, S], BF16, tag=f"ont{c}", name=f"ont{c}")
                   for c in range(NT)]

            # ---- post-attention tiles + helpers (emitted inside the last
            # head's attention loop so wo/LN1/ff1 overlap it) ----
            r1 = [acts.tile([128, S], BF16, tag=f"r1_{d0}", name=f"r1_{d0}")
                  for d0 in range(ND)]
            x1z = [acts.tile([128, S], BF16, tag=f"x1z{d0}", name=f"x1z{d0}")
                   for d0 in range(ND)]
            x1bD = acts.tile([128, 2, S], FP8, tag="x1bD", name="x1bD")
            hD = acts.tile([128, NF, S], FP8, tag="hD", name="hD")
            r2 = [acts.tile([128, S], BF16, tag=f"r2_{d0}", name=f"r2_{d0}")
                  for d0 in range(ND)]
            outT = [acts.tile([128, S], F32, tag=f"out{d0}", name=f"out{d0}")
                    for d0 in range(ND)]

            def ln_stats(j, src):
                """mean/meansq stats for chunk j via matmuls; squares on
                DVE (bf16 in/out SBUF => 2x mode)."""
                cols = slice(j * SC, (j + 1) * SC)
                sq = [work.tile([128, SC], BF16, tag=f"sq{d0}",
                                name=f"sq{d0}_{j}") for d0 in range(ND)]
                for d0 in range(ND):
                    nc.vector.tensor_mul(out=sq[d0][:], in0=src[d0][:, cols],
                                         in1=src[d0][:, cols])
                mup = psZ.tile([128, SC], F32, tag="z", name="mup")
                for d0 in range(ND):
                    nc.tensor.matmul(mup[:], invd_bf[:], src[d0][:, cols],
                                     start=(d0 == 0), stop=(d0 == ND - 1))
                m2p = psO.tile([128, SC], F32, tag="o", name="m2p")
                for d0 in range(ND):
                    nc.tensor.matmul(m2p[:], invd_bf[:], sq[d0][:],
                                     start=(d0 == 0), stop=(d0 == ND - 1))
                return mup, m2p

            def ln_rows(stats):
                """Shared row math; recip needs f32, applies get a bf16
                rstd twin so they hit the DVE 2x mode."""
                mup, m2p = stats
                musq = work.tile([128, SC], F32, tag="musq", name="musq")
                nc.scalar.activation(out=musq[:], in_=mup[:], func=AF.Square)
                var = work.tile([128, SC], F32, tag="var", name="var")
                nc.vector.tensor_sub(out=var[:], in0=m2p[:], in1=musq[:])
                sd = work.tile([128, SC], F32, tag="sd", name="sd")
                nc.scalar.activation(out=sd[:], in_=var[:], func=AF.Sqrt,
                                     bias=eps_t[:])
                rstd = work.tile([128, SC], F32, tag="rstd", name="rstd")
                nc.vector.reciprocal_approx_fast(out=rstd[:], in_=sd[:])
                rstdb = work.tile([128, SC], BF16, tag="rstdb", name="rstdb")
                nc.scalar.activation(out=rstdb[:], in_=rstd[:], func=AF.Copy)
                mr = work.tile([128, SC], BF16, tag="mr", name="mr")
                nc.vector.tensor_mul(out=mr[:], in0=mup[:], in1=rstd[:])
                return rstdb, mr

            def ln1_chain(j, stats):
                """z = (r1-mu)*rstd into x1z (bf16) + x1bD (fp8 twin).
                d0=0 applies on DVE (2x), d0=1 on Pool, twins on ACT."""
                cols = slice(j * SC, (j + 1) * SC)
                rstd, mr = ln_rows(stats)
                for d0 in range(ND):
                    eng = nc.vector if d0 == 0 else nc.gpsimd
                    t = work.tile([128, SC], BF16, tag=f"lnt{d0}",
                                  name=f"lnt{d0}")
                    eng.tensor_mul(out=t[:], in0=r1[d0][:, cols],
                                   in1=rstd[:])
                    eng.tensor_sub(out=x1z[d0][:, cols], in0=t[:],
                                   in1=mr[:])
                    nc.scalar.activation(out=x1bD[:, d0, cols],
                                         in_=x1z[d0][:, cols], func=AF.Copy)

            def ln2_chain(j, stats):
                """Direct-form LN2 apply: out = r2*(g2*rstd) - (mr*g2-b2);
                d0=0 on DVE (2x stt/ts), d0=1 on Pool; f32 out + DMA."""
                cols = slice(j * SC, (j + 1) * SC)
                rstd, mr = ln_rows(stats)
                for d0 in range(ND):
                    # scalar-Ptr ops only exist on DVE/ACT; Pool takes the
                    # plain final sub for d0=1
                    t = work.tile([128, SC], BF16, tag=f"l2t{d0}",
                                  name=f"l2t{d0}")
                    nc.vector.scalar_tensor_tensor(
                        out=t[:], in0=r2[d0][:, cols], scalar=ln2_g[d0][:],
                        in1=rstd[:], op0=ALU.mult, op1=ALU.mult)
                    bb = work.tile([128, SC], BF16, tag=f"l2b{d0}",
                                   name=f"l2b{d0}")
                    nc.vector.tensor_scalar(
                        out=bb[:], in0=mr[:], scalar1=ln2_g[d0][:],
                        scalar2=ln2_b[d0][:], op0=ALU.mult, op1=ALU.subtract)
                    eng = nc.vector if d0 == 0 else nc.gpsimd
                    eng.tensor_sub(out=outT[d0][:, cols], in0=t[:],
                                   in1=bb[:])
                    nc.sync.dma_start(out=out_d[d0][:, cols],
                                      in_=outT[d0][:, cols])

            def tail_work(j):
                """wo + LN1 + ff1 for chunk j; called right after the last
                head's normalize(j) so it overlaps remaining attention."""
                cols = slice(j * SC, (j + 1) * SC)
                for d0 in range(ND):
                    pp = psA.tile([128, SC], F32, tag="mm", name="womm")
                    for c in range(NT):
                        nc.tensor.matmul(
                            pp[:], woT[c][:, d0 * 128:(d0 + 1) * 128],
                            ont[c][:, cols], start=(c == 0),
                            stop=(c == NT - 1))
                    nc.vector.scalar_tensor_tensor(
                        out=r1[d0][:, cols], in0=pp[:], scalar=wo_b[d0][:],
                        in1=xT[d0][:, cols], op0=ALU.add, op1=ALU.add)
                ln1_chain(j, ln_stats(j, r1))
                for f0 in range(NF):
                    fp = psA.tile([128, SC], F32, tag="mm", name="ff1mm")
                    nc.tensor.matmul(
                        fp[:], ff1sw[:, f0, :],
                        x1bD[:, :, cols], start=True, stop=True,
                        perf_mode=DRS)
                    if f0 % 2 == 0:
                        nc.scalar.activation(out=hD[:, f0, cols], in_=fp[:],
                                             func=AF.Relu,
                                             bias=ff1b64[f0][:])
                    else:
                        nc.vector.tensor_scalar(
                            out=hD[:, f0, cols], in0=fp[:],
                            scalar1=ff1b64[f0][:], scalar2=0.0,
                            op0=ALU.add, op1=ALU.max)

            def tail_work2(j):
                """ff2 + residual + LN2 + output DMA for chunk j."""
                cols = slice(j * SC, (j + 1) * SC)
                for d0 in range(ND):
                    fp = psA.tile([128, SC], F32, tag="mm", name="ff2mm")
                    for c in range(NF // 2):
                        nc.tensor.matmul(
                            fp[:], ff2sw[:, c, d0, :],
                            hD[:, 2 * c:2 * c + 2, cols],
                            start=(c == 0), stop=(c == NF // 2 - 1),
                            perf_mode=DRS)
                    r2t = work.tile([128, SC], F32, tag="r2t", name="r2t")
                    nc.scalar.activation(out=r2t[:], in_=fp[:],
                                         func=AF.Identity,
                                         scale=FF2_SCALE, bias=ff2b_f[d0][:])
                    nc.vector.scalar_tensor_tensor(
                        out=r2[d0][:, cols], in0=x1z[d0][:, cols],
                        scalar=ln1_g[d0][:], in1=r2t[:],
                        op0=ALU.mult, op1=ALU.add)
                ln2_chain(j, ln_stats(j, r2))

            def make_gv(h):
                """Head h: gDsw [128,NT,256] = swi-packed 16*(M_h^T x);
                vDsw[u] [128,2,256] = swi-packed 16*v for key blocks
                (2u,2u+1). The producing matmuls read block-reversed rhs
                (xRD/wvRD) so these interleaving copies are plain
                positive-stride-2 writes."""
                gDsw = work.tile([128, NT, 256], FP8, tag="gD",
                                 name=f"gD{h}")
                vDsw = [work.tile([128, 2, 256], FP8, tag=f"vhD{u}",
                                  name=f"vhD{h}_{u}") for u in range(NT // 2)]

                def g_group(e0, j):
                    cols = slice(j * SC, (j + 1) * SC)
                    p = psA.tile([128, SC], F32, tag="mm", name="gmm")
                    nc.tensor.matmul(
                        p[:], MDsw[h][:, e0, :], xRD[:, :, cols],
                        start=True, stop=True, perf_mode=DRS)
                    if e0 == 0:
                        nc.scalar.activation(
                            out=gDsw[:, 4 * j:4 * (j + 1), e0::2],
                            in_=p[:].rearrange("p (b c) -> p b c", b=4),
                            func=AF.Copy, scale=G_SCALE)
                    else:
                        nc.vector.tensor_scalar(
                            out=gDsw[:, 4 * j:4 * (j + 1), e0::2],
                            in0=p[:].rearrange("p (b c) -> p b c", b=4),
                            scalar1=G_SCALE, scalar2=None, op0=ALU.mult)

                def v_group(t0):
                    p = psA.tile([128, SC], F32, tag="mm", name="vmm")
                    nc.tensor.matmul(
                        p[:, :E], xSW[:, t0, :],
                        wvRD[:, :, h * E:(h + 1) * E],
                        start=True, stop=True, perf_mode=DRS)
                    nc.vector.tensor_scalar(
                        out=vDsw[t0 // 2][:, :, (t0 % 2)::2],
                        in0=p[:, :E].rearrange("p (b c) -> p b c", b=2),
                        scalar1=V_SCALE, scalar2=None, op0=ALU.mult)

                thunks = []
                for e0 in range(2):
                    thunks.append(lambda e0=e0: g_group(e0, 0))
                for t0 in range(4):
                    thunks.append(lambda t0=t0: v_group(t0))
                for e0 in range(2):
                    thunks.append(lambda e0=e0: g_group(e0, 1))
                for t0 in range(4, NT):
                    thunks.append(lambda t0=t0: v_group(t0))
                return gDsw, vDsw, thunks

            cur = make_gv(0)
            for t in cur[2]:
                t()

            for h in range(H):
                gDsw, vDsw, _ = cur
                nxt = make_gv(h + 1) if h + 1 < H else None
                pending = list(nxt[2]) if nxt else []
                n_iters = 12
                it = 0
                done = 0
                zp = [psZ.tile([128, SC], F32, tag="z", name="z")
                      for j in range(NJ)]
                op = [[psO.tile([128, SC], F32, tag="o", name="o")
                       for _ in range(2)] for j in range(NJ)]
                for j in range(NJ):
                    kmax = 4 * j + 4
                    npair = kmax // 2
                    pend = []

                    def emit_zo(item):
                        jj, u, ekp, offp, w = item
                        last = (u == (4 * jj + 4) // 2 - 1)
                        nc.tensor.matmul(
                            zp[jj][:, offp:offp + w], onesSW[:],
                            ekp[:, :, offp:offp + w],
                            start=(u == 0), stop=last,
                            perf_mode=DRS, skip_group_check=True)
                        for e0 in range(2):
                            nc.tensor.matmul(
                                op[jj][e0][:, offp:offp + w],
                                vDsw[u][:, e0, :],
                                ekp[:, :, offp:offp + w],
                                start=(u == 0), stop=last,
                                perf_mode=DRS, skip_group_check=True)

                    ekp = None
                    offp = 0
                    for k in range(kmax):
                        start_col = max(SC * j, 128 * k)
                        off = start_col - SC * j
                        w = SC - off
                        u, parity = k // 2, k % 2
                        if parity == 0:
                            ekp = work.tile([128, 2, SC], FP8,
                                            tag=f"ek{u % 3}", name=f"ek{k}")
                            offp = off
                        sp = psA.tile([128, SC], F32, tag="mm", name="smm")
                        nc.tensor.matmul(
                            sp[:, off:off + w],
                            gDsw[:, k, :],
                            xD[:, :, start_col:start_col + w],
                            start=True, stop=True, perf_mode=DRS)
                        nc.scalar.activation(
                            out=ekp[:, parity, off:off + w],
                            in_=sp[:, off:off + w], func=AF.Exp,
                            scale=EXP_SCALE)
                        if 128 * k >= SC * j:
                            # diagonal block: zero the upper triangle
                            # (s < t) in place on Pool
                            nc.gpsimd.affine_select(
                                out=ekp[:, parity, off:off + 128],
                                in_=ekp[:, parity, off:off + 128],
                                compare_op=ALU.is_ge, fill=0.0,
                                base=0, pattern=[[1, 128]],
                                channel_multiplier=-1)
                        if parity == 1:
                            if off > offp:
                                # second block of the pair starts later:
                                # zero its below-range strip
                                nc.gpsimd.memset(ekp[:, 1, offp:off], 0.0)
                            pend.append((j, u, ekp, offp, SC - offp))
                            if len(pend) > 2:
                                emit_zo(pend.pop(0))
                        # stream next head's projections into this head's
                        # attention so head boundaries carry no stall
                        it += 1
                        want = (len(pending) * it + n_iters - 1) // n_iters \
                            if pending else 0
                        while done < want:
                            pending[done]()
                            done += 1
                    for item in pend:
                        emit_zo(item)
                    # normalize: ONT = op * (1/Z); Z replicated on all
                    # partitions by the onesD matmul
                    zb = work.tile([128, SC], F32, tag="zb", name="zb")
                    nc.vector.reciprocal_approx_fast(out=zb[:], in_=zp[j][:])
                    cols = slice(j * SC, (j + 1) * SC)
                    for e0 in range(2):
                        nc.vector.tensor_mul(
                            out=ont[h * 2 + e0][:, cols], in0=op[j][e0][:],
                            in1=zb[:])
                    if h == H - 1:
                        tail_work(j)
                while done < len(pending):
                    pending[done]()
                    done += 1
                if nxt:
                    cur = nxt

            for j in range(NJ):
                tail_work2(j)


    nc.compile()
    return nc


def _np_reference(x, attention_mask, wq, wk, wv, wo_w, wo_b, ln1_g, ln1_b,
                  ff1_w, ff1_b, ff2_w, ff2_b, ln2_g, ln2_b):
    """Numpy fallback (only used if attention_mask has zeros)."""
    def ln(t, g, b):
        mu = t.mean(-1, keepdims=True)
        var = t.var(-1, keepdims=True)
        return (t - mu) / np.sqrt(var + LN_EPS) * g + b
    Bn, Sn, Dn = x.shape
    q = np.einsum('bsd,hed->bhse', x, wq)
    k = np.einsum('bsd,hed->bhse', x, wk)
    v = np.einsum('bsd,hed->bhse', x, wv)
    sc = np.einsum('bhse,bhte->bhst', q, k) / np.sqrt(np.float32(Dn))
    idx = np.arange(Sn)
    causal = idx[None, :] > idx[:, None]
    m = attention_mask.astype(bool)
    valid = m[:, None, :] & m[:, :, None]
    cond = causal[None] | ~valid
    sc = np.where(cond[:, None], -np.inf, sc)
    sc = sc - np.nanmax(np.where(np.isinf(sc), np.nan, sc), axis=-1,
                        keepdims=True)
    e = np.exp(sc)
    e = np.where(np.isnan(e), 0.0, e)
    att = e / np.maximum(e.sum(-1, keepdims=True), 1e-30)
    ho = np.einsum('bhst,bhte->bhse', att, v)
    cat = np.transpose(ho, (0, 2, 1, 3)).reshape(Bn, Sn, -1)
    mh = cat @ wo_w.T + wo_b
    x1 = ln(x + mh, ln1_g, ln1_b)
    hh = np.maximum(x1 @ ff1_w.T + ff1_b, 0.0)
    ff = hh @ ff2_w.T + ff2_b
    return ln(x1 + ff, ln2_g, ln2_b).astype(np.float32)


def _pack2(a):
    """[2*128, N...] -> [128, 2, N...] (partition-major packing of a
    256-deep contraction: rhs layout for DoubleRow modes)."""
    n = a.shape[0] // 128
    return np.ascontiguousarray(
        a.reshape(n, 128, *a.shape[1:]).transpose(
            1, 0, *range(2, a.ndim + 1)))


def _swi(Wi):
    """SwInterleave weight packing for one 128-col weight block.
    Wi: logical [2(i), 128(p), 128(c)] -> [128, 256] with
    out[p, 2t+i] = Wi[i, p, 127-t] (pairs interleaved, columns
    reversed — the layout DoubleRowSwInterleave's LDWEIGHTS expects)."""
    rev = Wi[:, :, ::-1]
    out = np.empty((128, 256), Wi.dtype)
    out[:, 0::2] = rev[0]
    out[:, 1::2] = rev[1]
    return out


def _rev_blocks(a):
    """Reverse columns within each 128-col block of the last axis."""
    sh = a.shape
    return np.ascontiguousarray(
        a.reshape(*sh[:-1], sh[-1] // 128, 128)[..., ::-1].reshape(sh))


def _prep_inputs(inputs):
    bf = ml_dtypes.bfloat16
    f8 = ml_dtypes.float8_e4m3
    x = np.asarray(inputs["x"], np.float32)
    wq = np.asarray(inputs["wq"], np.float32)
    wk = np.asarray(inputs["wk"], np.float32)
    wv = np.asarray(inputs["wv"], np.float32)

    # per-head M = Wq^T Wk (f32 on host), swi-packed per d-output block
    M = np.einsum('hed,hef->hdf', wq, wk)  # [H, D(d), D(d')]
    MDsw = np.empty((H, 128, 2, 256), f8)
    for h in range(H):
        MT = (M[h].T * 1024.0).astype(f8).reshape(2, 128, D)  # [i, p, d]
        for e0 in range(2):
            MDsw[h, :, e0, :] = _swi(MT[:, :, e0 * 128:(e0 + 1) * 128])

    # v-proj rhs: wv^T with e-columns reversed per 128-chunk, so the v
    # PSUM comes out column-reversed and the vDsw interleave write is a
    # positive-stride-2 AP
    wvT = np.ascontiguousarray(wv.transpose(2, 0, 1).reshape(D, HE))
    wvRD = _pack2(_rev_blocks(wvT * 64.0).astype(f8))

    woT = np.ascontiguousarray(np.asarray(inputs["wo_w"], np.float32).T
                               ).astype(bf).reshape(NT, 128, D)
    ff1w = np.asarray(inputs["ff1_w"], np.float32)
    ln1_g = np.asarray(inputs["ln1_g"], np.float32)
    ln1_b = np.asarray(inputs["ln1_b"], np.float32)
    # LN1 gamma/beta folded into ff1 (ff1 consumes the pure normalized z)
    ff1gf = ff1w * ln1_g[None, :]
    ff1b_f = (np.asarray(inputs["ff1_b"], np.float32) + ff1w @ ln1_b) * 64.0
    ff1T = (np.ascontiguousarray(ff1gf.T) * 64.0).astype(f8)  # [D, FF]
    ff1TT = ff1T.reshape(2, 128, FF)
    ff1sw = np.empty((128, NF, 256), f8)
    for f0 in range(NF):
        ff1sw[:, f0, :] = _swi(ff1TT[:, :, f0 * 128:(f0 + 1) * 128])
    ff2T = (np.ascontiguousarray(np.asarray(inputs["ff2_w"], np.float32).T)
            * 32.0).astype(f8)  # [FF, D]
    ff2TT = ff2T.reshape(NF // 2, 2, 128, D)
    ff2sw = np.empty((128, NF // 2, 2, 256), f8)
    for cc in range(NF // 2):
        for d0 in range(2):
            ff2sw[:, cc, d0, :] = _swi(
                ff2TT[cc][:, :, d0 * 128:(d0 + 1) * 128])

    shared = dict(
        MDsw=MDsw, wvRD=wvRD, woT=woT, ff1sw=ff1sw, ff2sw=ff2sw,
        wo_b=np.asarray(inputs["wo_b"], np.float32).reshape(ND, 128, 1),
        ff1b64=ff1b_f.reshape(NF, 128, 1),
        ff2b_f=(np.asarray(inputs["ff2_b"], np.float32)
                + np.asarray(inputs["ln1_b"], np.float32)
                ).reshape(ND, 128, 1),
        ln1_g=np.asarray(inputs["ln1_g"], np.float32).reshape(ND, 128, 1),
        ln1_b=np.asarray(inputs["ln1_b"], np.float32).reshape(ND, 128, 1),
        ln2_g=np.asarray(inputs["ln2_g"], np.float32).reshape(ND, 128, 1),
        ln2_b=np.asarray(inputs["ln2_b"], np.float32).reshape(ND, 128, 1),
    )
    in_maps = []
    for b in range(B):
        xTb = np.ascontiguousarray(x[b].T)  # [D, S]
        x8 = (xTb * 8.0).astype(f8)
        xsw = np.empty((128, NT, 256), f8)
        x8r = x8.reshape(2, 128, S)
        for t0 in range(NT):
            xsw[:, t0, :] = _swi(x8r[:, :, t0 * 128:(t0 + 1) * 128])
        m = dict(shared)
        m["xT"] = xTb.reshape(ND, 128, S)
        m["xD"] = _pack2(x8)
        m["xRD"] = _pack2(_rev_blocks(xTb * 8.0).astype(f8))
        m["xSW"] = xsw
        in_maps.append(m)
    return in_maps


def run_sharded(inputs, trace=False, trace_kwargs=None):
    if "nc" not in _CACHE:
        _CACHE["nc"] = _build()
    nc = _CACHE["nc"]
    in_maps = _prep_inputs(inputs)
    res = run_bass_kernel_spmd(nc, in_maps, list(range(N_CORES)), trace=trace,
                               **(trace_kwargs or {}))
    outs = []
    for b in range(B):
        r = np.asarray(res.results[b]["out"], np.float32).reshape(D, S)
        outs.append(r.T)
    return np.stack(outs), res


def kernel(**inputs) -> np.ndarray:
    mask = np.asarray(inputs["attention_mask"])
    if not np.all(mask != 0):
        return _np_reference(**{k: np.asarray(v) for k, v in inputs.items()})
    out, _ = run_sharded(inputs, trace=False)
    return out


# revision 17
# speedup vs baseline: 1.0876x; 1.0876x over previous
"""Trainium2 Bass kernel for nn_DecoderBlock (B=8, S=1024, D=256, H=4 heads
of full width 256, FF=1024).

Strategy: pure data parallelism — B=8 batch elements across 8 NeuronCores,
zero collectives. Per core, one full decoder block in "transposed" activation
layout (features on SBUF partitions, tokens on the free dim).

Perf tricks over the bf16 baseline:
- Scores use the host-precomputed per-head M = Wq^T Wk: scores = x^T M x,
  so only ONE projection g = M^T x per head is materialized (no separate
  q/k), halving projection matmuls and PSUM->SBUF copies.
- fp8e4 (TRN E4M3, max +-240) with perf_mode=DoubleRowSwInterleave for
  the attention core (scores, softmax denominator Z, att@V), the g/v
  projections and both FFN matmuls: 2 fp8 weights/cell => 0.5 PE cycles
  per output column, 256-deep contraction per instruction. (Plain
  DoubleRow hangs the exec unit on this silicon; SwInterleave with
  host/on-chip interleaved+column-reversed weights works.) On-chip
  produced weights (g, att@V's v) are written through positive-stride-2
  APs; their column reversal comes free by feeding the PRODUCING matmul
  block-reversed rhs data (xRD / wvRD). Scale management keeps every
  fp8 operand in the E4M3 normal range (x*8, M*1024, wv*64, ff1*64,
  ff2*32; exp() values are O(1) naturally).
- Causal masking via gpsimd (Pool) affine_select directly on the fp8 exp
  tiles; Pool also takes the LayerNorm applies + squares (it has no PSUM
  port, so PSUM-reading ops stay on ACT/DVE).
- LN mean stats matmuls read the f32 residual via a float32r bitcast
  (1 cycle/col at free>=256) so no bf16 twin of the residual is written.
- ACT activation-table discipline: attention uses only Exp; everything
  after uses {Square, Rsqrt, Copy, Relu} which share one table
  (reciprocal_sqrt_and_small) => a single table swap per kernel.

The attention_mask input is all ones per the problem spec (causal mask
only); if a mask with zeros ever shows up, we fall back to numpy.
"""

import numpy as np
import ml_dtypes

import concourse.bass as bass
import concourse.mybir as mybir
import concourse.tile as tile
from concourse import bacc
from concourse.bass_utils import run_bass_kernel_spmd

F32 = mybir.dt.float32
F32R = mybir.dt.float32r
BF16 = mybir.dt.bfloat16
FP8 = mybir.dt.float8e4
AF = mybir.ActivationFunctionType
ALU = mybir.AluOpType
DRS = mybir.MatmulPerfMode.DoubleRowSwInterleave

N_CORES = 8
B, S, D, H, E, HE, FF = 8, 1024, 256, 4, 256, 1024, 1024
SC = 512          # token (free-dim) chunk
NJ = S // SC      # 2 chunks of tokens
ND = D // 128     # 2 partition chunks of features
NF = FF // 128    # 8 partition chunks of ff features
NT = S // 128     # 8 partition chunks of tokens
LN_EPS = 1e-5
# scaling: x*8, M*4096 -> g psum = 32768*g, gD = 16*g (copy scale 1/2048)
# scores psum = gD*xD = 128 * (x^T M x) = 128*16*scores_ref -> exp scale:
EXP_SCALE = 1.0 / 2048.0
G_SCALE = 1.0 / 512.0    # g psum (8x*1024M = 8192g) -> gD (16*g)
V_SCALE = 1.0 / 32.0     # v psum (8x * 64wv = 512v) -> vhD (16*v)
FF2_SCALE = 1.0 / 2048.0  # ff2 psum (64h * 32w = 2048*out) -> out

_CACHE = {}


def _build():
    nc = bacc.Bacc("TRN2", target_bir_lowering=False, debug=False,
                   num_devices=N_CORES)

    # ---- DRAM parameters (per-core shard + replicated weights) ----
    xD_d = nc.dram_tensor("xD", [128, 2, S], FP8, kind="ExternalInput")
    xRD_d = nc.dram_tensor("xRD", [128, 2, S], FP8, kind="ExternalInput")
    xSW_d = nc.dram_tensor("xSW", [128, NT, 2 * 128], FP8,
                           kind="ExternalInput")
    MD_d = nc.dram_tensor("MDsw", [H, 128, 2, 2 * 128], FP8,
                          kind="ExternalInput")
    wvD_d = nc.dram_tensor("wvRD", [H, 128, 2, E], FP8, kind="ExternalInput")
    xT_d = nc.dram_tensor("xT", [ND, 128, S], F32, kind="ExternalInput")
    woT_d = nc.dram_tensor("woT", [NT, 128, D], BF16, kind="ExternalInput")
    ff1D_d = nc.dram_tensor("ff1sw", [128, NF, 2 * 128], FP8,
                            kind="ExternalInput")
    ff2D_d = nc.dram_tensor("ff2sw", [128, NF // 2, 2, 2 * 128], FP8,
                            kind="ExternalInput")
    wo_b_d = nc.dram_tensor("wo_b", [ND, 128, 1], F32, kind="ExternalInput")
    ff1b64_d = nc.dram_tensor("ff1b64", [NF, 128, 1], F32, kind="ExternalInput")
    ff2_b_d = nc.dram_tensor("ff2b_f", [ND, 128, 1], F32, kind="ExternalInput")
    ln1_g_d = nc.dram_tensor("ln1_g", [ND, 128, 1], F32, kind="ExternalInput")
    ln1_b_d = nc.dram_tensor("ln1_b", [ND, 128, 1], F32, kind="ExternalInput")
    ln2_g_d = nc.dram_tensor("ln2_g", [ND, 128, 1], F32, kind="ExternalInput")
    ln2_b_d = nc.dram_tensor("ln2_b", [ND, 128, 1], F32, kind="ExternalInput")
    out_d = nc.dram_tensor("out", [ND, 128, S], F32, kind="ExternalOutput")

    with tile.TileContext(nc) as tc:
        with tc.tile_pool(name="consts", bufs=1) as consts, \
             tc.tile_pool(name="acts", bufs=1) as acts, \
             tc.tile_pool(name="work", bufs=2) as work, \
             tc.tile_pool(name="psA", bufs=3, space="PSUM") as psA, \
             tc.tile_pool(name="psO", bufs=3, space="PSUM") as psO, \
             tc.tile_pool(name="psZ", bufs=2, space="PSUM") as psZ:

            def loadc(dram, shape, dt, name, src=None):
                t = consts.tile(shape, dt, tag=name, name=name)
                nc.sync.dma_start(out=t[:], in_=dram[:] if src is None else src)
                return t

            # DMA in strict first-use order, with x tensors split in
            # halves, so head 0's first matmuls start ~1.5us in
            xRD = consts.tile([128, 2, S], FP8, tag="xRD", name="xRD")
            xD = consts.tile([128, 2, S], FP8, tag="xD", name="xD")
            nc.sync.dma_start(out=xRD[:, :, :SC], in_=xRD_d[:, :, :SC])
            MDsw = [None] * H
            MDsw[0] = loadc(MD_d, [128, 2, 256], FP8, "MDsw0", src=MD_d[0])
            nc.sync.dma_start(out=xD[:, :, :SC], in_=xD_d[:, :, :SC])
            xSW = loadc(xSW_d, [128, NT, 256], FP8, "xSW")
            wvRD = [None] * H
            wvRD[0] = loadc(wvD_d, [128, 2, E], FP8, "wvRD0", src=wvD_d[0])
            nc.sync.dma_start(out=xRD[:, :, SC:], in_=xRD_d[:, :, SC:])
            nc.sync.dma_start(out=xD[:, :, SC:], in_=xD_d[:, :, SC:])
            for h in range(1, H):
                MDsw[h] = loadc(MD_d, [128, 2, 256], FP8, f"MDsw{h}",
                                src=MD_d[h])
                wvRD[h] = loadc(wvD_d, [128, 2, E], FP8, f"wvRD{h}",
                                src=wvD_d[h])
            xT = [loadc(xT_d, [128, S], F32, f"xT{i}", src=xT_d[i])
                  for i in range(ND)]
            woT = [loadc(woT_d, [128, D], BF16, f"woT{i}", src=woT_d[i])
                   for i in range(NT)]
            ff1sw = loadc(ff1D_d, [128, NF, 256], FP8, "ff1sw")
            ff2sw = loadc(ff2D_d, [128, NF // 2, 2, 256], FP8, "ff2sw")

            def loadb(dram, n, name):
                return [loadc(dram, [128, 1], F32, f"{name}{i}", src=dram[i])
                        for i in range(n)]
            wo_b = loadb(wo_b_d, ND, "wo_b")
            ff1b64 = loadb(ff1b64_d, NF, "ff1b64")
            ff2b_f = loadb(ff2_b_d, ND, "ff2b_f")
            ln1_g = loadb(ln1_g_d, ND, "ln1_g")
            ln1_b = loadb(ln1_b_d, ND, "ln1_b")
            ln2_g = loadb(ln2_g_d, ND, "ln2_g")
            ln2_b = loadb(ln2_b_d, ND, "ln2_b")

            # constants
            onesSW = consts.tile([128, 256], FP8, tag="onesSW", name="onesSW")
            nc.vector.memset(onesSW[:], 16.0)  # matches the 16*v scale
            invd_bf = consts.tile([128, 128], BF16, tag="invdb", name="invdb")
            nc.vector.memset(invd_bf[:], 1.0 / D)
            eps_t = consts.tile([128, 1], F32, tag="eps", name="eps")
            nc.vector.memset(eps_t[:], LN_EPS)
            warm_bf = consts.tile([128, 128], BF16, tag="warmw", name="warmw")
            nc.vector.memset(warm_bf[:], 0.0)

            # PE warmup: dummy matmuls (no DMA dependency) keep the HAM
            # clock gate ramping while the input DMAs land.
            def warm_pe(n):
                for _ in range(n):
                    wp = psA.tile([128, SC], F32, tag="mm", name="warm")
                    nc.tensor.matmul(wp[:, :128], warm_bf[:], warm_bf[:],
                                     start=True, stop=True)

            warm_pe(22)

            # ---- attention ----
            # ONT [HE, S] normalized head outputs (bf16) for the wo matmul
            ont = [acts.tile([128, S], BF16, tag=f"ont{c}", name=f"ont{c}")
                   for c in range(NT)]

            # ---- post-attention tiles + helpers (emitted inside the last
            # head's attention loop so wo/LN1/ff1 overlap it) ----
            r1 = [acts.tile([128, S], BF16, tag=f"r1_{d0}", name=f"r1_{d0}")
                  for d0 in range(ND)]
            x1z = [acts.tile([128, S], BF16, tag=f"x1z{d0}", name=f"x1z{d0}")
                   for d0 in range(ND)]
            x1bD = acts.tile([128, 2, S], FP8, tag="x1bD", name="x1bD")
            hD = acts.tile([128, NF, S], FP8, tag="hD", name="hD")
            r2 = [acts.tile([128, S], BF16, tag=f"r2_{d0}", name=f"r2_{d0}")
                  for d0 in range(ND)]
            outT = [acts.tile([128, S], F32, tag=f"out{d0}", name=f"out{d0}")
                    for d0 in range(ND)]

            def ln_stats(j, src):
                """mean/meansq stats for chunk j via matmuls; squares on
                DVE (bf16 in/out SBUF => 2x mode)."""
                cols = slice(j * SC, (j + 1) * SC)
                sq = [work.tile([128, SC], BF16, tag=f"sq{d0}",
                                name=f"sq{d0}_{j}") for d0 in range(ND)]
                for d0 in range(ND):
                    nc.vector.tensor_mul(out=sq[d0][:], in0=src[d0][:, cols],
                                         in1=src[d0][:, cols])
                mup = psZ.tile([128, SC], F32, tag="z", name="mup")
                for d0 in range(ND):
                    nc.tensor.matmul(mup[:], invd_bf[:], src[d0][:, cols],
                                     start=(d0 == 0), stop=(d0 == ND - 1))
                m2p = psO.tile([128, SC], F32, tag="o", name="m2p")
                for d0 in range(ND):
                    nc.tensor.matmul(m2p[:], invd_bf[:], sq[d0][:],
                                     start=(d0 == 0), stop=(d0 == ND - 1))
                return mup, m2p

            def ln_rows(stats):
                """Shared row math; recip needs f32, applies get a bf16
                rstd twin so they hit the DVE 2x mode."""
                mup, m2p = stats
                musq = work.tile([128, SC], F32, tag="musq", name="musq")
                nc.scalar.activation(out=musq[:], in_=mup[:], func=AF.Square)
                var = work.tile([128, SC], F32, tag="var", name="var")
                nc.vector.tensor_sub(out=var[:], in0=m2p[:], in1=musq[:])
                sd = work.tile([128, SC], F32, tag="sd", name="sd")
                nc.scalar.activation(out=sd[:], in_=var[:], func=AF.Sqrt,
                                     bias=eps_t[:])
                rstd = work.tile([128, SC], F32, tag="rstd", name="rstd")
                nc.vector.reciprocal_approx_fast(out=rstd[:], in_=sd[:])
                rstdb = work.tile([128, SC], BF16, tag="rstdb", name="rstdb")
                nc.scalar.activation(out=rstdb[:], in_=rstd[:], func=AF.Copy)
                mr = work.tile([128, SC], BF16, tag="mr", name="mr")
                nc.vector.tensor_mul(out=mr[:], in0=mup[:], in1=rstd[:])
                return rstdb, mr

            def ln1_chain(j, stats):
                """z = (r1-mu)*rstd into x1z (bf16) + x1bD (fp8 twin).
                d0=0 applies on DVE (2x), d0=1 on Pool, twins on ACT."""
                cols = slice(j * SC, (j + 1) * SC)
                rstd, mr = ln_rows(stats)
                for d0 in range(ND):
                    eng = nc.vector if d0 == 0 else nc.gpsimd
                    t = work.tile([128, SC], BF16, tag=f"lnt{d0}",
                                  name=f"lnt{d0}")
                    eng.tensor_mul(out=t[:], in0=r1[d0][:, cols],
                                   in1=rstd[:])
                    eng.tensor_sub(out=x1z[d0][:, cols], in0=t[:],
                                   in1=mr[:])
                    nc.scalar.activation(out=x1bD[:, d0, cols],
                                         in_=x1z[d0][:, cols], func=AF.Copy)

            def ln2_chain(j, stats):
                """Direct-form LN2 apply: out = r2*(g2*rstd) - (mr*g2-b2);
                d0=0 on DVE (2x stt/ts), d0=1 on Pool; f32 out + DMA."""
                cols = slice(j * SC, (j + 1) * SC)
                rstd, mr = ln_rows(stats)
                for d0 in range(ND):
                    # scalar-Ptr ops only exist on DVE/ACT; Pool takes the
                    # plain final sub for d0=1
                    t = work.tile([128, SC], BF16, tag=f"l2t{d0}",
                                  name=f"l2t{d0}")
                    nc.vector.scalar_tensor_tensor(
                        out=t[:], in0=r2[d0][:, cols], scalar=ln2_g[d0][:],
                        in1=rstd[:], op0=ALU.mult, op1=ALU.mult)
                    bb = work.tile([128, SC], BF16, tag=f"l2b{d0}",
                                   name=f"l2b{d0}")
                    nc.vector.tensor_scalar(
                        out=bb[:], in0=mr[:], scalar1=ln2_g[d0][:],
                        scalar2=ln2_b[d0][:], op0=ALU.mult, op1=ALU.subtract)
                    eng = nc.vector if d0 == 0 else nc.gpsimd
                    eng.tensor_sub(out=outT[d0][:, cols], in0=t[:],
                                   in1=bb[:])
                    nc.sync.dma_start(out=out_d[d0][:, cols],
                                      in_=outT[d0][:, cols])

            def tail_work(j):
                """wo + LN1 + ff1 for chunk j; called right after the last
                head's normalize(j) so it overlaps remaining attention."""
                cols = slice(j * SC, (j + 1) * SC)
                for d0 in range(ND):
                    pp = psA.tile([128, SC], F32, tag="mm", name="womm")
                    for c in range(NT):
                        nc.tensor.matmul(
                            pp[:], woT[c][:, d0 * 128:(d0 + 1) * 128],
                            ont[c][:, cols], start=(c == 0),
                            stop=(c == NT - 1))
                    nc.vector.scalar_tensor_tensor(
                        out=r1[d0][:, cols], in0=pp[:], scalar=wo_b[d0][:],
                        in1=xT[d0][:, cols], op0=ALU.add, op1=ALU.add)
                ln1_chain(j, ln_stats(j, r1))

            def ff1_work(j):
                cols = slice(j * SC, (j + 1) * SC)
                for f0 in range(NF):
                    fp = psA.tile([128, SC], F32, tag="mm", name="ff1mm")
                    nc.tensor.matmul(
                        fp[:], ff1sw[:, f0, :],
                        x1bD[:, :, cols], start=True, stop=True,
                        perf_mode=DRS)
                    if f0 % 2 == 0:
                        nc.scalar.activation(out=hD[:, f0, cols], in_=fp[:],
                                             func=AF.Relu,
                                             bias=ff1b64[f0][:])
                    else:
                        nc.vector.tensor_scalar(
                            out=hD[:, f0, cols], in0=fp[:],
                            scalar1=ff1b64[f0][:], scalar2=0.0,
                            op0=ALU.add, op1=ALU.max)

            def ff2_work(j):
                """ff2 matmuls + residual for chunk j."""
                cols = slice(j * SC, (j + 1) * SC)
                for d0 in range(ND):
                    fp = psA.tile([128, SC], F32, tag="mm", name="ff2mm")
                    for c in range(NF // 2):
                        nc.tensor.matmul(
                            fp[:], ff2sw[:, c, d0, :],
                            hD[:, 2 * c:2 * c + 2, cols],
                            start=(c == 0), stop=(c == NF // 2 - 1),
                            perf_mode=DRS)
                    r2t = work.tile([128, SC], F32, tag="r2t", name="r2t")
                    nc.scalar.activation(out=r2t[:], in_=fp[:],
                                         func=AF.Identity,
                                         scale=FF2_SCALE, bias=ff2b_f[d0][:])
                    nc.vector.scalar_tensor_tensor(
                        out=r2[d0][:, cols], in0=x1z[d0][:, cols],
                        scalar=ln1_g[d0][:], in1=r2t[:],
                        op0=ALU.mult, op1=ALU.add)

            def make_gv(h):
                """Head h: gDsw [128,NT,256] = swi-packed 16*(M_h^T x);
                vDsw[u] [128,2,256] = swi-packed 16*v for key blocks
                (2u,2u+1). The producing matmuls read block-reversed rhs
                (xRD/wvRD) so these interleaving copies are plain
                positive-stride-2 writes."""
                gDsw = work.tile([128, NT, 256], FP8, tag="gD",
                                 name=f"gD{h}")
                vDsw = [work.tile([128, 2, 256], FP8, tag=f"vhD{u}",
                                  name=f"vhD{h}_{u}") for u in range(NT // 2)]

                def g_group(e0, j):
                    cols = slice(j * SC, (j + 1) * SC)
                    p = psA.tile([128, SC], F32, tag="mm", name="gmm")
                    nc.tensor.matmul(
                        p[:], MDsw[h][:, e0, :], xRD[:, :, cols],
                        start=True, stop=True, perf_mode=DRS)
                    if e0 == 0:
                        nc.scalar.activation(
                            out=gDsw[:, 4 * j:4 * (j + 1), e0::2],
                            in_=p[:].rearrange("p (b c) -> p b c", b=4),
                            func=AF.Copy, scale=G_SCALE)
                    else:
                        nc.vector.tensor_scalar(
                            out=gDsw[:, 4 * j:4 * (j + 1), e0::2],
                            in0=p[:].rearrange("p (b c) -> p b c", b=4),
                            scalar1=G_SCALE, scalar2=None, op0=ALU.mult)

                def v_group(t0):
                    p = psA.tile([128, SC], F32, tag="mm", name="vmm")
                    nc.tensor.matmul(
                        p[:, :E], xSW[:, t0, :],
                        wvRD[h][:],
                        start=True, stop=True, perf_mode=DRS)
                    nc.vector.tensor_scalar(
                        out=vDsw[t0 // 2][:, :, (t0 % 2)::2],
                        in0=p[:, :E].rearrange("p (b c) -> p b c", b=2),
                        scalar1=V_SCALE, scalar2=None, op0=ALU.mult)

                thunks = []
                for e0 in range(2):
                    thunks.append(lambda e0=e0: g_group(e0, 0))
                for t0 in range(4):
                    thunks.append(lambda t0=t0: v_group(t0))
                for e0 in range(2):
                    thunks.append(lambda e0=e0: g_group(e0, 1))
                for t0 in range(4, NT):
                    thunks.append(lambda t0=t0: v_group(t0))
                return gDsw, vDsw, thunks

            cur = make_gv(0)
            for t in cur[2]:
                t()

            for h in range(H):
                gDsw, vDsw, _ = cur
                nxt = make_gv(h + 1) if h + 1 < H else None
                pending = list(nxt[2]) if nxt else []
                n_iters = 12
                it = 0
                done = 0
                zp = [psZ.tile([128, SC], F32, tag="z", name="z")
                      for j in range(NJ)]
                op = [[psO.tile([128, SC], F32, tag="o", name="o")
                       for _ in range(2)] for j in range(NJ)]
                for j in range(NJ):
                    kmax = 4 * j + 4
                    npair = kmax // 2
                    pend = []

                    def emit_zo(item):
                        jj, u, ekp, offp, w = item
                        last = (u == (4 * jj + 4) // 2 - 1)
                        nc.tensor.matmul(
                            zp[jj][:, offp:offp + w], onesSW[:],
                            ekp[:, :, offp:offp + w],
                            start=(u == 0), stop=last,
                            perf_mode=DRS, skip_group_check=True)
                        for e0 in range(2):
                            nc.tensor.matmul(
                                op[jj][e0][:, offp:offp + w],
                                vDsw[u][:, e0, :],
                                ekp[:, :, offp:offp + w],
                                start=(u == 0), stop=last,
                                perf_mode=DRS, skip_group_check=True)

                    ekp = None
                    offp = 0
                    for k in range(kmax):
                        start_col = max(SC * j, 128 * k)
                        off = start_col - SC * j
                        w = SC - off
                        u, parity = k // 2, k % 2
                        if parity == 0:
                            ekp = work.tile([128, 2, SC], FP8,
                                            tag=f"ek{u % 3}", name=f"ek{k}")
                            offp = off
                        sp = psA.tile([128, SC], F32, tag="mm", name="smm")
                        nc.tensor.matmul(
                            sp[:, off:off + w],
                            gDsw[:, k, :],
                            xD[:, :, start_col:start_col + w],
                            start=True, stop=True, perf_mode=DRS)
                        nc.scalar.activation(
                            out=ekp[:, parity, off:off + w],
                            in_=sp[:, off:off + w], func=AF.Exp,
                            scale=EXP_SCALE)
                        if 128 * k >= SC * j:
                            # diagonal block: zero the upper triangle
                            # (s < t) in place on Pool
                            nc.gpsimd.affine_select(
                                out=ekp[:, parity, off:off + 128],
                                in_=ekp[:, parity, off:off + 128],
                                compare_op=ALU.is_ge, fill=0.0,
                                base=0, pattern=[[1, 128]],
                                channel_multiplier=-1)
                        if parity == 1:
                            if off > offp:
                                # second block of the pair starts later:
                                # zero its below-range strip
                                nc.gpsimd.memset(ekp[:, 1, offp:off], 0.0)
                            pend.append((j, u, ekp, offp, SC - offp))
                            if len(pend) > 2:
                                emit_zo(pend.pop(0))
                        # stream next head's projections into this head's
                        # attention so head boundaries carry no stall
                        it += 1
                        want = (len(pending) * it + n_iters - 1) // n_iters \
                            if pending else 0
                        while done < want:
                            pending[done]()
                            done += 1
                    for item in pend:
                        emit_zo(item)
                    # normalize: ONT = op * (1/Z); Z replicated on all
                    # partitions by the onesD matmul
                    zb = work.tile([128, SC], F32, tag="zb", name="zb")
                    nc.vector.reciprocal_approx_fast(out=zb[:], in_=zp[j][:])
                    cols = slice(j * SC, (j + 1) * SC)
                    for e0 in range(2):
                        nc.vector.tensor_mul(
                            out=ont[h * 2 + e0][:, cols], in0=op[j][e0][:],
                            in1=zb[:])
                    if h == H - 1:
                        tail_work(j)
                while done < len(pending):
                    pending[done]()
                    done += 1
                if nxt:
                    cur = nxt

            # post-attention: PE sequence chosen so matmul groups are
            # mostly dep-ready when the PE reaches them; LN chains run on
            # ACT/DVE/Pool behind it; LN2 of the last chunk at 256-wide
            # halves to shorten the exposed serial tail
            ff1_work(0)
            ff1_work(1)
            ff2_work(0)
            st20 = ln_stats(0, r2)
            ff2_work(1)
            ln2_chain(0, st20)
            ln2_chain(1, ln_stats(1, r2))


    nc.compile()
    return nc


def _np_reference(x, attention_mask, wq, wk, wv, wo_w, wo_b, ln1_g, ln1_b,
                  ff1_w, ff1_b, ff2_w, ff2_b, ln2_g, ln2_b):
    """Numpy fallback (only used if attention_mask has zeros)."""
    def ln(t, g, b):
        mu = t.mean(-1, keepdims=True)
        var = t.var(-1, keepdims=True)
        return (t - mu) / np.sqrt(var + LN_EPS) * g + b
    Bn, Sn, Dn = x.shape
    q = np.einsum('bsd,hed->bhse', x, wq)
    k = np.einsum('bsd,hed->bhse', x, wk)
    v = np.einsum('bsd,hed->bhse', x, wv)
    sc = np.einsum('bhse,bhte->bhst', q, k) / np.sqrt(np.float32(Dn))
    idx = np.arange(Sn)
    causal = idx[None, :] > idx[:, None]
    m = attention_mask.astype(bool)
    valid = m[:, None, :] & m[:, :, None]
    cond = causal[None] | ~valid
    sc = np.where(cond[:, None], -np.inf, sc)
    sc = sc - np.nanmax(np.where(np.isinf(sc), np.nan, sc), axis=-1,
                        keepdims=True)
    e = np.exp(sc)
    e = np.where(np.isnan(e), 0.0, e)
    att = e / np.maximum(e.sum(-1, keepdims=True), 1e-30)
    ho = np.einsum('bhst,bhte->bhse', att, v)
    cat = np.transpose(ho, (0, 2, 1, 3)).reshape(Bn, Sn, -1)
    mh = cat @ wo_w.T + wo_b
    x1 = ln(x + mh, ln1_g, ln1_b)
    hh = np.maximum(x1 @ ff1_w.T + ff1_b, 0.0)
    ff = hh @ ff2_w.T + ff2_b
    return ln(x1 + ff, ln2_g, ln2_b).astype(np.float32)


def _pack2(a):
    """[2*128, N...] -> [128, 2, N...] (partition-major packing of a
    256-deep contraction: rhs layout for DoubleRow modes)."""
    n = a.shape[0] // 128
    return np.ascontiguousarray(
        a.reshape(n, 128, *a.shape[1:]).transpose(
            1, 0, *range(2, a.ndim + 1)))


def _swi(Wi):
    """SwInterleave weight packing for one 128-col weight block.
    Wi: logical [2(i), 128(p), 128(c)] -> [128, 256] with
    out[p, 2t+i] = Wi[i, p, 127-t] (pairs interleaved, columns
    reversed — the layout DoubleRowSwInterleave's LDWEIGHTS expects)."""
    rev = Wi[:, :, ::-1]
    out = np.empty((128, 256), Wi.dtype)
    out[:, 0::2] = rev[0]
    out[:, 1::2] = rev[1]
    return out


def _rev_blocks(a):
    """Reverse columns within each 128-col block of the last axis."""
    sh = a.shape
    return np.ascontiguousarray(
        a.reshape(*sh[:-1], sh[-1] // 128, 128)[..., ::-1].reshape(sh))


def _prep_inputs(inputs):
    bf = ml_dtypes.bfloat16
    f8 = ml_dtypes.float8_e4m3
    x = np.asarray(inputs["x"], np.float32)
    wq = np.asarray(inputs["wq"], np.float32)
    wk = np.asarray(inputs["wk"], np.float32)
    wv = np.asarray(inputs["wv"], np.float32)

    # per-head M = Wq^T Wk (f32 on host), swi-packed per d-output block
    M = np.einsum('hed,hef->hdf', wq, wk)  # [H, D(d), D(d')]
    MDsw = np.empty((H, 128, 2, 256), f8)
    for h in range(H):
        MT = (M[h].T * 1024.0).astype(f8).reshape(2, 128, D)  # [i, p, d]
        for e0 in range(2):
            MDsw[h, :, e0, :] = _swi(MT[:, :, e0 * 128:(e0 + 1) * 128])

    # v-proj rhs: wv^T with e-columns reversed per 128-chunk, so the v
    # PSUM comes out column-reversed and the vDsw interleave write is a
    # positive-stride-2 AP
    wvT = np.ascontiguousarray(wv.transpose(2, 0, 1).reshape(D, HE))
    wvRD = np.ascontiguousarray(
        _pack2(_rev_blocks(wvT * 64.0).astype(f8))
        .reshape(128, 2, H, E).transpose(2, 0, 1, 3))

    woT = np.ascontiguousarray(np.asarray(inputs["wo_w"], np.float32).T
                               ).astype(bf).reshape(NT, 128, D)
    ff1w = np.asarray(inputs["ff1_w"], np.float32)
    ln1_g = np.asarray(inputs["ln1_g"], np.float32)
    ln1_b = np.asarray(inputs["ln1_b"], np.float32)
    # LN1 gamma/beta folded into ff1 (ff1 consumes the pure normalized z)
    ff1gf = ff1w * ln1_g[None, :]
    ff1b_f = (np.asarray(inputs["ff1_b"], np.float32) + ff1w @ ln1_b) * 64.0
    ff1T = (np.ascontiguousarray(ff1gf.T) * 64.0).astype(f8)  # [D, FF]
    ff1TT = ff1T.reshape(2, 128, FF)
    ff1sw = np.empty((128, NF, 256), f8)
    for f0 in range(NF):
        ff1sw[:, f0, :] = _swi(ff1TT[:, :, f0 * 128:(f0 + 1) * 128])
    ff2T = (np.ascontiguousarray(np.asarray(inputs["ff2_w"], np.float32).T)
            * 32.0).astype(f8)  # [FF, D]
    ff2TT = ff2T.reshape(NF // 2, 2, 128, D)
    ff2sw = np.empty((128, NF // 2, 2, 256), f8)
    for cc in range(NF // 2):
        for d0 in range(2):
            ff2sw[:, cc, d0, :] = _swi(
                ff2TT[cc][:, :, d0 * 128:(d0 + 1) * 128])

    shared = dict(
        MDsw=MDsw, wvRD=wvRD, woT=woT, ff1sw=ff1sw, ff2sw=ff2sw,
        wo_b=np.asarray(inputs["wo_b"], np.float32).reshape(ND, 128, 1),
        ff1b64=ff1b_f.reshape(NF, 128, 1),
        ff2b_f=(np.asarray(inputs["ff2_b"], np.float32)
                + np.asarray(inputs["ln1_b"], np.float32)
                ).reshape(ND, 128, 1),
        ln1_g=np.asarray(inputs["ln1_g"], np.float32).reshape(ND, 128, 1),
        ln1_b=np.asarray(inputs["ln1_b"], np.float32).reshape(ND, 128, 1),
        ln2_g=np.asarray(inputs["ln2_g"], np.float32).reshape(ND, 128, 1),
        ln2_b=np.asarray(inputs["ln2_b"], np.float32).reshape(ND, 128, 1),
    )
    in_maps = []
    for b in range(B):
        xTb = np.ascontiguousarray(x[b].T)  # [D, S]
        x8 = (xTb * 8.0).astype(f8)
        xsw = np.empty((128, NT, 256), f8)
        x8r = x8.reshape(2, 128, S)
        for t0 in range(NT):
            xsw[:, t0, :] = _swi(x8r[:, :, t0 * 128:(t0 + 1) * 128])
        m = dict(shared)
        m["xT"] = xTb.reshape(ND, 128, S)
        m["xD"] = _pack2(x8)
        m["xRD"] = _pack2(_rev_blocks(xTb * 8.0).astype(f8))
        m["xSW"] = xsw
        in_maps.append(m)
    return in_maps


def run_sharded(inputs, trace=False, trace_kwargs=None):
    if "nc" not in _CACHE:
        _CACHE["nc"] = _build()
    nc = _CACHE["nc"]
    in_maps = _prep_inputs(inputs)
    res = run_bass_kernel_spmd(nc, in_maps, list(range(N_CORES)), trace=trace,
                               **(trace_kwargs or {}))
    outs = []
    for b in range(B):
        r = np.asarray(res.results[b]["out"], np.float32).reshape(D, S)
        outs.append(r.T)
    return np.stack(outs), res


def kernel(**inputs) -> np.ndarray:
    mask = np.asarray(inputs["attention_mask"])
    if not np.all(mask != 0):
        return _np_reference(**{k: np.asarray(v) for k, v in inputs.items()})
    out, _ = run_sharded(inputs, trace=False)
    return out


# revision 19
# speedup vs baseline: 1.1831x; 1.0879x over previous
"""Trainium2 Bass kernel for nn_DecoderBlock (B=8, S=1024, D=256, H=4 heads
of full width 256, FF=1024).

Strategy: pure data parallelism — B=8 batch elements across 8 NeuronCores,
zero collectives. Per core, one full decoder block in "transposed" activation
layout (features on SBUF partitions, tokens on the free dim).

Perf tricks over the bf16 baseline:
- Scores use the host-precomputed per-head M = Wq^T Wk: scores = x^T M x,
  so only ONE projection g = M^T x per head is materialized (no separate
  q/k), halving projection matmuls and PSUM->SBUF copies.
- fp8e4 (TRN E4M3, max +-240) with perf_mode=DoubleRowSwInterleave for
  the attention core (scores, softmax denominator Z, att@V), the g/v
  projections and both FFN matmuls: 2 fp8 weights/cell => 0.5 PE cycles
  per output column, 256-deep contraction per instruction. (Plain
  DoubleRow hangs the exec unit on this silicon; SwInterleave with
  host/on-chip interleaved+column-reversed weights works.) On-chip
  produced weights (g, att@V's v) are written through positive-stride-2
  APs; their column reversal comes free by feeding the PRODUCING matmul
  block-reversed rhs data (xRD / wvRD). Scale management keeps every
  fp8 operand in the E4M3 normal range (x*8, M*1024, wv*64, ff1*64,
  ff2*32; exp() values are O(1) naturally).
- Causal masking via gpsimd (Pool) affine_select directly on the fp8 exp
  tiles; Pool also takes the LayerNorm applies + squares (it has no PSUM
  port, so PSUM-reading ops stay on ACT/DVE).
- LN mean stats matmuls read the f32 residual via a float32r bitcast
  (1 cycle/col at free>=256) so no bf16 twin of the residual is written.
- ACT activation-table discipline: attention uses only Exp; everything
  after uses {Square, Rsqrt, Copy, Relu} which share one table
  (reciprocal_sqrt_and_small) => a single table swap per kernel.

The attention_mask input is all ones per the problem spec (causal mask
only); if a mask with zeros ever shows up, we fall back to numpy.
"""

import numpy as np
import ml_dtypes

import concourse.bass as bass
import concourse.mybir as mybir
import concourse.tile as tile
from concourse import bacc
from concourse.bass_utils import run_bass_kernel_spmd

F32 = mybir.dt.float32
F32R = mybir.dt.float32r
BF16 = mybir.dt.bfloat16
FP8 = mybir.dt.float8e4
AF = mybir.ActivationFunctionType
ALU = mybir.AluOpType
DRS = mybir.MatmulPerfMode.DoubleRowSwInterleave

N_CORES = 8
B, S, D, H, E, HE, FF = 8, 1024, 256, 4, 256, 1024, 1024
SC = 512          # token (free-dim) chunk
NJ = S // SC      # 2 chunks of tokens
ND = D // 128     # 2 partition chunks of features
NF = FF // 128    # 8 partition chunks of ff features
NT = S // 128     # 8 partition chunks of tokens
LN_EPS = 1e-5
# scaling: x*8, M*4096 -> g psum = 32768*g, gD = 16*g (copy scale 1/2048)
# scores psum = gD*xD = 128 * (x^T M x) = 128*16*scores_ref -> exp scale:
EXP_SCALE = 1.0 / 2048.0
G_SCALE = 1.0 / 512.0    # g psum (8x*1024M = 8192g) -> gD (16*g)
V_SCALE = 1.0 / 32.0     # v psum (8x * 64wv = 512v) -> vhD (16*v)
FF2_SCALE = 1.0 / 2048.0  # ff2 psum (64h * 32w = 2048*out) -> out

_CACHE = {}


def _build():
    nc = bacc.Bacc("TRN2", target_bir_lowering=False, debug=False,
                   num_devices=N_CORES)

    # ---- DRAM parameters (per-core shard + replicated weights) ----
    xD_d = nc.dram_tensor("xD", [128, 2, S], FP8, kind="ExternalInput")
    xRD_d = nc.dram_tensor("xRD", [128, 2, S], FP8, kind="ExternalInput")
    xSW_d = nc.dram_tensor("xSW", [128, NT, 2 * 128], FP8,
                           kind="ExternalInput")
    MD_d = nc.dram_tensor("MDsw", [H, 128, 2, 2 * 128], FP8,
                          kind="ExternalInput")
    wvD_d = nc.dram_tensor("wvRD", [H, 128, 2, E], FP8, kind="ExternalInput")
    xT_d = nc.dram_tensor("xT", [ND, 128, S], F32, kind="ExternalInput")
    woT_d = nc.dram_tensor("woT", [NT, 128, D], BF16, kind="ExternalInput")
    ff1D_d = nc.dram_tensor("ff1sw", [128, NF, 2 * 128], FP8,
                            kind="ExternalInput")
    ff2D_d = nc.dram_tensor("ff2sw", [128, NF // 2, 2, 2 * 128], FP8,
                            kind="ExternalInput")
    wo_b_d = nc.dram_tensor("wo_b", [ND, 128, 1], F32, kind="ExternalInput")
    ff1b64_d = nc.dram_tensor("ff1b64", [NF, 128, 1], F32, kind="ExternalInput")
    ff2_b_d = nc.dram_tensor("ff2b_f", [ND, 128, 1], F32, kind="ExternalInput")
    ln1_g_d = nc.dram_tensor("ln1_g", [ND, 128, 1], F32, kind="ExternalInput")
    ln1_b_d = nc.dram_tensor("ln1_b", [ND, 128, 1], F32, kind="ExternalInput")
    ln2_g_d = nc.dram_tensor("ln2_g", [ND, 128, 1], F32, kind="ExternalInput")
    ln2_b_d = nc.dram_tensor("ln2_b", [ND, 128, 1], F32, kind="ExternalInput")
    out_d = nc.dram_tensor("out", [ND, 128, S], BF16, kind="ExternalOutput")

    with tile.TileContext(nc) as tc:
        with tc.tile_pool(name="consts", bufs=1) as consts, \
             tc.tile_pool(name="acts", bufs=1) as acts, \
             tc.tile_pool(name="work", bufs=2) as work, \
             tc.tile_pool(name="psA", bufs=3, space="PSUM") as psA, \
             tc.tile_pool(name="psO", bufs=3, space="PSUM") as psO, \
             tc.tile_pool(name="psZ", bufs=2, space="PSUM") as psZ:

            def loadc(dram, shape, dt, name, src=None):
                t = consts.tile(shape, dt, tag=name, name=name)
                nc.sync.dma_start(out=t[:], in_=dram[:] if src is None else src)
                return t

            # DMA in strict first-use order, with x tensors split in
            # halves, so head 0's first matmuls start ~1.5us in
            xRD = consts.tile([128, 2, S], FP8, tag="xRD", name="xRD")
            xD = consts.tile([128, 2, S], FP8, tag="xD", name="xD")
            nc.sync.dma_start(out=xRD[:, :, :SC], in_=xRD_d[:, :, :SC])
            MDsw = [None] * H
            MDsw[0] = loadc(MD_d, [128, 2, 256], FP8, "MDsw0", src=MD_d[0])
            nc.sync.dma_start(out=xD[:, :, :SC], in_=xD_d[:, :, :SC])
            xSW = loadc(xSW_d, [128, NT, 256], FP8, "xSW")
            wvRD = [None] * H
            wvRD[0] = loadc(wvD_d, [128, 2, E], FP8, "wvRD0", src=wvD_d[0])
            nc.sync.dma_start(out=xRD[:, :, SC:], in_=xRD_d[:, :, SC:])
            nc.sync.dma_start(out=xD[:, :, SC:], in_=xD_d[:, :, SC:])
            for h in range(1, H):
                MDsw[h] = loadc(MD_d, [128, 2, 256], FP8, f"MDsw{h}",
                                src=MD_d[h])
                wvRD[h] = loadc(wvD_d, [128, 2, E], FP8, f"wvRD{h}",
                                src=wvD_d[h])
            xT = [loadc(xT_d, [128, S], F32, f"xT{i}", src=xT_d[i])
                  for i in range(ND)]
            woT = [loadc(woT_d, [128, D], BF16, f"woT{i}", src=woT_d[i])
                   for i in range(NT)]
            ff1sw = loadc(ff1D_d, [128, NF, 256], FP8, "ff1sw")
            ff2sw = loadc(ff2D_d, [128, NF // 2, 2, 256], FP8, "ff2sw")

            def loadb(dram, n, name):
                return [loadc(dram, [128, 1], F32, f"{name}{i}", src=dram[i])
                        for i in range(n)]
            wo_b = loadb(wo_b_d, ND, "wo_b")
            ff1b64 = loadb(ff1b64_d, NF, "ff1b64")
            ff2b_f = loadb(ff2_b_d, ND, "ff2b_f")
            ln1_g = loadb(ln1_g_d, ND, "ln1_g")
            ln1_b = loadb(ln1_b_d, ND, "ln1_b")
            ln2_g = loadb(ln2_g_d, ND, "ln2_g")
            ln2_b = loadb(ln2_b_d, ND, "ln2_b")

            # constants
            onesSW = consts.tile([128, 256], FP8, tag="onesSW", name="onesSW")
            nc.vector.memset(onesSW[:], 16.0)  # matches the 16*v scale
            invd_bf = consts.tile([128, 128], BF16, tag="invdb", name="invdb")
            nc.vector.memset(invd_bf[:], 1.0 / D)
            eps_t = consts.tile([128, 1], F32, tag="eps", name="eps")
            nc.vector.memset(eps_t[:], LN_EPS)
            warm_bf = consts.tile([128, 128], BF16, tag="warmw", name="warmw")
            nc.vector.memset(warm_bf[:], 0.0)

            # PE warmup: dummy matmuls (no DMA dependency) keep the HAM
            # clock gate ramping while the input DMAs land.
            def warm_pe(n):
                for _ in range(n):
                    wp = psA.tile([128, SC], F32, tag="mm", name="warm")
                    nc.tensor.matmul(wp[:, :128], warm_bf[:], warm_bf[:],
                                     start=True, stop=True)

            warm_pe(22)

            # ---- attention ----
            # ONT [HE, S] normalized head outputs (bf16) for the wo matmul
            ont = [acts.tile([128, S], BF16, tag=f"ont{c}", name=f"ont{c}")
                   for c in range(NT)]

            # ---- post-attention tiles + helpers (emitted inside the last
            # head's attention loop so wo/LN1/ff1 overlap it) ----
            r1 = [acts.tile([128, S], BF16, tag=f"r1_{d0}", name=f"r1_{d0}")
                  for d0 in range(ND)]
            x1z = [acts.tile([128, S], BF16, tag=f"x1z{d0}", name=f"x1z{d0}")
                   for d0 in range(ND)]
            x1bD = acts.tile([128, 2, S], FP8, tag="x1bD", name="x1bD")
            hD = acts.tile([128, NF, S], FP8, tag="hD", name="hD")
            r2 = [acts.tile([128, S], BF16, tag=f"r2_{d0}", name=f"r2_{d0}")
                  for d0 in range(ND)]
            outT = [acts.tile([128, S], BF16, tag=f"out{d0}", name=f"out{d0}")
                    for d0 in range(ND)]

            def ln_stats(j, src):
                """mean/meansq stats for chunk j via matmuls; squares on
                DVE (bf16 in/out SBUF => 2x mode)."""
                cols = slice(j * SC, (j + 1) * SC)
                sq = [work.tile([128, SC], BF16, tag=f"sq{d0}",
                                name=f"sq{d0}_{j}") for d0 in range(ND)]
                for d0 in range(ND):
                    nc.vector.tensor_mul(out=sq[d0][:], in0=src[d0][:, cols],
                                         in1=src[d0][:, cols])
                mup = psZ.tile([128, SC], F32, tag="z", name="mup")
                for d0 in range(ND):
                    nc.tensor.matmul(mup[:], invd_bf[:], src[d0][:, cols],
                                     start=(d0 == 0), stop=(d0 == ND - 1))
                m2p = psO.tile([128, SC], F32, tag="o", name="m2p")
                for d0 in range(ND):
                    nc.tensor.matmul(m2p[:], invd_bf[:], sq[d0][:],
                                     start=(d0 == 0), stop=(d0 == ND - 1))
                return mup, m2p

            def ln_rows(stats):
                """Shared row math; recip needs f32, applies get a bf16
                rstd twin so they hit the DVE 2x mode."""
                mup, m2p = stats
                musq = work.tile([128, SC], F32, tag="musq", name="musq")
                nc.scalar.activation(out=musq[:], in_=mup[:], func=AF.Square)
                var = work.tile([128, SC], F32, tag="var", name="var")
                nc.vector.tensor_sub(out=var[:], in0=m2p[:], in1=musq[:])
                sd = work.tile([128, SC], F32, tag="sd", name="sd")
                nc.scalar.activation(out=sd[:], in_=var[:], func=AF.Sqrt,
                                     bias=eps_t[:])
                rstd = work.tile([128, SC], F32, tag="rstd", name="rstd")
                nc.vector.reciprocal_approx_fast(out=rstd[:], in_=sd[:])
                rstdb = work.tile([128, SC], BF16, tag="rstdb", name="rstdb")
                nc.scalar.activation(out=rstdb[:], in_=rstd[:], func=AF.Copy)
                mr = work.tile([128, SC], BF16, tag="mr", name="mr")
                nc.vector.tensor_mul(out=mr[:], in0=mup[:], in1=rstd[:])
                return rstdb, mr

            def ln1_chain(j, stats):
                """z = (r1-mu)*rstd into x1z (bf16) + x1bD (fp8 twin).
                d0=0 applies on DVE (2x), d0=1 on Pool, twins on ACT."""
                cols = slice(j * SC, (j + 1) * SC)
                rstd, mr = ln_rows(stats)
                for d0 in range(ND):
                    t = work.tile([128, SC], BF16, tag=f"lnt{d0}",
                                  name=f"lnt{d0}")
                    nc.vector.tensor_mul(out=t[:], in0=r1[d0][:, cols],
                                         in1=rstd[:])
                    nc.vector.tensor_sub(out=x1z[d0][:, cols], in0=t[:],
                                         in1=mr[:])
                    nc.scalar.activation(out=x1bD[:, d0, cols],
                                         in_=x1z[d0][:, cols], func=AF.Copy)

            def ln2_chain(j, stats):
                """Direct-form LN2 apply: out = r2*(g2*rstd) - (mr*g2-b2);
                d0=0 on DVE (2x stt/ts), d0=1 on Pool; f32 out + DMA."""
                cols = slice(j * SC, (j + 1) * SC)
                rstd, mr = ln_rows(stats)
                for d0 in range(ND):
                    # scalar-Ptr ops only exist on DVE/ACT; Pool takes the
                    # plain final sub for d0=1
                    t = work.tile([128, SC], BF16, tag=f"l2t{d0}",
                                  name=f"l2t{d0}")
                    nc.vector.scalar_tensor_tensor(
                        out=t[:], in0=r2[d0][:, cols], scalar=ln2_g[d0][:],
                        in1=rstd[:], op0=ALU.mult, op1=ALU.mult)
                    bb = work.tile([128, SC], BF16, tag=f"l2b{d0}",
                                   name=f"l2b{d0}")
                    nc.vector.tensor_scalar(
                        out=bb[:], in0=mr[:], scalar1=ln2_g[d0][:],
                        scalar2=ln2_b[d0][:], op0=ALU.mult, op1=ALU.subtract)
                    nc.vector.tensor_sub(out=outT[d0][:, cols], in0=t[:],
                                           in1=bb[:])
                    nc.sync.dma_start(out=out_d[d0][:, cols],
                                      in_=outT[d0][:, cols])

            def tail_work(j):
                """wo + LN1 + ff1 for chunk j; called right after the last
                head's normalize(j) so it overlaps remaining attention."""
                cols = slice(j * SC, (j + 1) * SC)
                for d0 in range(ND):
                    pp = psA.tile([128, SC], F32, tag="mm", name="womm")
                    for c in range(NT):
                        nc.tensor.matmul(
                            pp[:], woT[c][:, d0 * 128:(d0 + 1) * 128],
                            ont[c][:, cols], start=(c == 0),
                            stop=(c == NT - 1))
                    nc.vector.scalar_tensor_tensor(
                        out=r1[d0][:, cols], in0=pp[:], scalar=wo_b[d0][:],
                        in1=xT[d0][:, cols], op0=ALU.add, op1=ALU.add)

            def ff1_work(j):
                cols = slice(j * SC, (j + 1) * SC)
                for f0 in range(NF):
                    fp = psA.tile([128, SC], F32, tag="mm", name="ff1mm")
                    nc.tensor.matmul(
                        fp[:], ff1sw[:, f0, :],
                        x1bD[:, :, cols], start=True, stop=True,
                        perf_mode=DRS)
                    if f0 % 2 == 0:
                        nc.scalar.activation(out=hD[:, f0, cols], in_=fp[:],
                                             func=AF.Relu,
                                             bias=ff1b64[f0][:])
                    else:
                        nc.vector.tensor_scalar(
                            out=hD[:, f0, cols], in0=fp[:],
                            scalar1=ff1b64[f0][:], scalar2=0.0,
                            op0=ALU.add, op1=ALU.max)

            def ff2_work(j):
                """ff2 matmuls + residual for chunk j."""
                cols = slice(j * SC, (j + 1) * SC)
                for d0 in range(ND):
                    fp = psA.tile([128, SC], F32, tag="mm", name="ff2mm")
                    for c in range(NF // 2):
                        nc.tensor.matmul(
                            fp[:], ff2sw[:, c, d0, :],
                            hD[:, 2 * c:2 * c + 2, cols],
                            start=(c == 0), stop=(c == NF // 2 - 1),
                            perf_mode=DRS)
                    r2t = work.tile([128, SC], F32, tag="r2t", name="r2t")
                    nc.scalar.activation(out=r2t[:], in_=fp[:],
                                         func=AF.Identity,
                                         scale=FF2_SCALE, bias=ff2b_f[d0][:])
                    nc.vector.scalar_tensor_tensor(
                        out=r2[d0][:, cols], in0=x1z[d0][:, cols],
                        scalar=ln1_g[d0][:], in1=r2t[:],
                        op0=ALU.mult, op1=ALU.add)

            def make_gv(h):
                """Head h: gDsw [128,NT,256] = swi-packed 16*(M_h^T x);
                vDsw[u] [128,2,256] = swi-packed 16*v for key blocks
                (2u,2u+1). The producing matmuls read block-reversed rhs
                (xRD/wvRD) so these interleaving copies are plain
                positive-stride-2 writes."""
                gDsw = work.tile([128, NT, 256], FP8, tag="gD",
                                 name=f"gD{h}")
                vDsw = [work.tile([128, 2, 256], FP8, tag=f"vhD{u}",
                                  name=f"vhD{h}_{u}") for u in range(NT // 2)]

                def g_group(e0, j):
                    cols = slice(j * SC, (j + 1) * SC)
                    p = psA.tile([128, SC], F32, tag="mm", name="gmm")
                    nc.tensor.matmul(
                        p[:], MDsw[h][:, e0, :], xRD[:, :, cols],
                        start=True, stop=True, perf_mode=DRS)
                    if e0 == 0:
                        nc.scalar.activation(
                            out=gDsw[:, 4 * j:4 * (j + 1), e0::2],
                            in_=p[:].rearrange("p (b c) -> p b c", b=4),
                            func=AF.Copy, scale=G_SCALE)
                    else:
                        nc.vector.tensor_scalar(
                            out=gDsw[:, 4 * j:4 * (j + 1), e0::2],
                            in0=p[:].rearrange("p (b c) -> p b c", b=4),
                            scalar1=G_SCALE, scalar2=None, op0=ALU.mult)

                def v_group(t0):
                    p = psA.tile([128, SC], F32, tag="mm", name="vmm")
                    nc.tensor.matmul(
                        p[:, :E], xSW[:, t0, :],
                        wvRD[h][:],
                        start=True, stop=True, perf_mode=DRS)
                    nc.vector.tensor_scalar(
                        out=vDsw[t0 // 2][:, :, (t0 % 2)::2],
                        in0=p[:, :E].rearrange("p (b c) -> p b c", b=2),
                        scalar1=V_SCALE, scalar2=None, op0=ALU.mult)

                thunks = []
                for e0 in range(2):
                    thunks.append(lambda e0=e0: g_group(e0, 0))
                for t0 in range(4):
                    thunks.append(lambda t0=t0: v_group(t0))
                for e0 in range(2):
                    thunks.append(lambda e0=e0: g_group(e0, 1))
                for t0 in range(4, NT):
                    thunks.append(lambda t0=t0: v_group(t0))
                return gDsw, vDsw, thunks

            cur = make_gv(0)
            for t in cur[2]:
                t()

            for h in range(H):
                gDsw, vDsw, _ = cur
                nxt = make_gv(h + 1) if h + 1 < H else None
                pending = list(nxt[2]) if nxt else []
                n_iters = 12
                it = 0
                done = 0
                zp = [psZ.tile([128, SC], F32, tag="z", name="z")
                      for j in range(NJ)]
                op = [[psO.tile([128, SC], F32, tag="o", name="o")
                       for _ in range(2)] for j in range(NJ)]
                for j in range(NJ):
                    kmax = 4 * j + 4
                    npair = kmax // 2
                    pend = []

                    def emit_zo(item):
                        jj, u, ekp, offp, w = item
                        last = (u == (4 * jj + 4) // 2 - 1)
                        nc.tensor.matmul(
                            zp[jj][:, offp:offp + w], onesSW[:],
                            ekp[:, :, offp:offp + w],
                            start=(u == 0), stop=last,
                            perf_mode=DRS, skip_group_check=True)
                        for e0 in range(2):
                            nc.tensor.matmul(
                                op[jj][e0][:, offp:offp + w],
                                vDsw[u][:, e0, :],
                                ekp[:, :, offp:offp + w],
                                start=(u == 0), stop=last,
                                perf_mode=DRS, skip_group_check=True)

                    ekp = None
                    offp = 0
                    for k in range(kmax):
                        start_col = max(SC * j, 128 * k)
                        off = start_col - SC * j
                        w = SC - off
                        u, parity = k // 2, k % 2
                        if parity == 0:
                            ekp = work.tile([128, 2, SC], FP8,
                                            tag=f"ek{u % 3}", name=f"ek{k}")
                            offp = off
                        sp = psA.tile([128, SC], F32, tag="mm", name="smm")
                        nc.tensor.matmul(
                            sp[:, off:off + w],
                            gDsw[:, k, :],
                            xD[:, :, start_col:start_col + w],
                            start=True, stop=True, perf_mode=DRS)
                        nc.scalar.activation(
                            out=ekp[:, parity, off:off + w],
                            in_=sp[:, off:off + w], func=AF.Exp,
                            scale=EXP_SCALE)
                        if 128 * k >= SC * j:
                            # diagonal block: zero the upper triangle
                            # (s < t) in place on Pool
                            nc.gpsimd.affine_select(
                                out=ekp[:, parity, off:off + 128],
                                in_=ekp[:, parity, off:off + 128],
                                compare_op=ALU.is_ge, fill=0.0,
                                base=0, pattern=[[1, 128]],
                                channel_multiplier=-1)
                        if parity == 1:
                            if off > offp:
                                # second block of the pair starts later:
                                # zero its below-range strip
                                nc.gpsimd.memset(ekp[:, 1, offp:off], 0.0)
                            pend.append((j, u, ekp, offp, SC - offp))
                            if len(pend) > 2:
                                emit_zo(pend.pop(0))
                        # stream next head's projections into this head's
                        # attention so head boundaries carry no stall
                        it += 1
                        want = (len(pending) * it + n_iters - 1) // n_iters \
                            if pending else 0
                        while done < want:
                            pending[done]()
                            done += 1
                    for item in pend:
                        emit_zo(item)
                    # normalize: ONT = op * (1/Z); Z replicated on all
                    # partitions by the onesD matmul
                    zb = work.tile([128, SC], F32, tag="zb", name="zb")
                    nc.vector.reciprocal_approx_fast(out=zb[:], in_=zp[j][:])
                    cols = slice(j * SC, (j + 1) * SC)
                    for e0 in range(2):
                        nc.vector.tensor_mul(
                            out=ont[h * 2 + e0][:, cols], in0=op[j][e0][:],
                            in1=zb[:])
                    if h == H - 1:
                        tail_work(j)
                while done < len(pending):
                    pending[done]()
                    done += 1
                if nxt:
                    cur = nxt

            # post-attention: stats first (PE, dep-ready), chains on
            # ACT/DVE behind warm-PE bridges that hold the clock at full
            # rate until the ff matmuls unblock
            st10 = ln_stats(0, r1)
            st11 = ln_stats(1, r1)
            ln1_chain(0, st10)
            ln1_chain(1, st11)
            warm_pe(16)
            ff1_work(0)
            ff1_work(1)
            ff2_work(0)
            st20 = ln_stats(0, r2)
            ff2_work(1)
            ln2_chain(0, st20)
            ln2_chain(1, ln_stats(1, r2))


    nc.compile()
    return nc


def _np_reference(x, attention_mask, wq, wk, wv, wo_w, wo_b, ln1_g, ln1_b,
                  ff1_w, ff1_b, ff2_w, ff2_b, ln2_g, ln2_b):
    """Numpy fallback (only used if attention_mask has zeros)."""
    def ln(t, g, b):
        mu = t.mean(-1, keepdims=True)
        var = t.var(-1, keepdims=True)
        return (t - mu) / np.sqrt(var + LN_EPS) * g + b
    Bn, Sn, Dn = x.shape
    q = np.einsum('bsd,hed->bhse', x, wq)
    k = np.einsum('bsd,hed->bhse', x, wk)
    v = np.einsum('bsd,hed->bhse', x, wv)
    sc = np.einsum('bhse,bhte->bhst', q, k) / np.sqrt(np.float32(Dn))
    idx = np.arange(Sn)
    causal = idx[None, :] > idx[:, None]
    m = attention_mask.astype(bool)
    valid = m[:, None, :] & m[:, :, None]
    cond = causal[None] | ~valid
    sc = np.where(cond[:, None], -np.inf, sc)
    sc = sc - np.nanmax(np.where(np.isinf(sc), np.nan, sc), axis=-1,
                        keepdims=True)
    e = np.exp(sc)
    e = np.where(np.isnan(e), 0.0, e)
    att = e / np.maximum(e.sum(-1, keepdims=True), 1e-30)
    ho = np.einsum('bhst,bhte->bhse', att, v)
    cat = np.transpose(ho, (0, 2, 1, 3)).reshape(Bn, Sn, -1)
    mh = cat @ wo_w.T + wo_b
    x1 = ln(x + mh, ln1_g, ln1_b)
    hh = np.maximum(x1 @ ff1_w.T + ff1_b, 0.0)
    ff = hh @ ff2_w.T + ff2_b
    return ln(x1 + ff, ln2_g, ln2_b).astype(np.float32)


def _pack2(a):
    """[2*128, N...] -> [128, 2, N...] (partition-major packing of a
    256-deep contraction: rhs layout for DoubleRow modes)."""
    n = a.shape[0] // 128
    return np.ascontiguousarray(
        a.reshape(n, 128, *a.shape[1:]).transpose(
            1, 0, *range(2, a.ndim + 1)))


def _swi(Wi):
    """SwInterleave weight packing for one 128-col weight block.
    Wi: logical [2(i), 128(p), 128(c)] -> [128, 256] with
    out[p, 2t+i] = Wi[i, p, 127-t] (pairs interleaved, columns
    reversed — the layout DoubleRowSwInterleave's LDWEIGHTS expects)."""
    rev = Wi[:, :, ::-1]
    out = np.empty((128, 256), Wi.dtype)
    out[:, 0::2] = rev[0]
    out[:, 1::2] = rev[1]
    return out


def _rev_blocks(a):
    """Reverse columns within each 128-col block of the last axis."""
    sh = a.shape
    return np.ascontiguousarray(
        a.reshape(*sh[:-1], sh[-1] // 128, 128)[..., ::-1].reshape(sh))


def _prep_inputs(inputs):
    bf = ml_dtypes.bfloat16
    f8 = ml_dtypes.float8_e4m3
    x = np.asarray(inputs["x"], np.float32)
    wq = np.asarray(inputs["wq"], np.float32)
    wk = np.asarray(inputs["wk"], np.float32)
    wv = np.asarray(inputs["wv"], np.float32)

    # per-head M = Wq^T Wk (f32 on host), swi-packed per d-output block
    M = np.einsum('hed,hef->hdf', wq, wk)  # [H, D(d), D(d')]
    MDsw = np.empty((H, 128, 2, 256), f8)
    for h in range(H):
        MT = (M[h].T * 1024.0).astype(f8).reshape(2, 128, D)  # [i, p, d]
        for e0 in range(2):
            MDsw[h, :, e0, :] = _swi(MT[:, :, e0 * 128:(e0 + 1) * 128])

    # v-proj rhs: wv^T with e-columns reversed per 128-chunk, so the v
    # PSUM comes out column-reversed and the vDsw interleave write is a
    # positive-stride-2 AP
    wvT = np.ascontiguousarray(wv.transpose(2, 0, 1).reshape(D, HE))
    wvRD = np.ascontiguousarray(
        _pack2(_rev_blocks(wvT * 64.0).astype(f8))
        .reshape(128, 2, H, E).transpose(2, 0, 1, 3))

    woT = np.ascontiguousarray(np.asarray(inputs["wo_w"], np.float32).T
                               ).astype(bf).reshape(NT, 128, D)
    ff1w = np.asarray(inputs["ff1_w"], np.float32)
    ln1_g = np.asarray(inputs["ln1_g"], np.float32)
    ln1_b = np.asarray(inputs["ln1_b"], np.float32)
    # LN1 gamma/beta folded into ff1 (ff1 consumes the pure normalized z)
    ff1gf = ff1w * ln1_g[None, :]
    ff1b_f = (np.asarray(inputs["ff1_b"], np.float32) + ff1w @ ln1_b) * 64.0
    ff1T = (np.ascontiguousarray(ff1gf.T) * 64.0).astype(f8)  # [D, FF]
    ff1TT = ff1T.reshape(2, 128, FF)
    ff1sw = np.empty((128, NF, 256), f8)
    for f0 in range(NF):
        ff1sw[:, f0, :] = _swi(ff1TT[:, :, f0 * 128:(f0 + 1) * 128])
    ff2T = (np.ascontiguousarray(np.asarray(inputs["ff2_w"], np.float32).T)
            * 32.0).astype(f8)  # [FF, D]
    ff2TT = ff2T.reshape(NF // 2, 2, 128, D)
    ff2sw = np.empty((128, NF // 2, 2, 256), f8)
    for cc in range(NF // 2):
        for d0 in range(2):
            ff2sw[:, cc, d0, :] = _swi(
                ff2TT[cc][:, :, d0 * 128:(d0 + 1) * 128])

    shared = dict(
        MDsw=MDsw, wvRD=wvRD, woT=woT, ff1sw=ff1sw, ff2sw=ff2sw,
        wo_b=np.asarray(inputs["wo_b"], np.float32).reshape(ND, 128, 1),
        ff1b64=ff1b_f.reshape(NF, 128, 1),
        ff2b_f=(np.asarray(inputs["ff2_b"], np.float32)
                + np.asarray(inputs["ln1_b"], np.float32)
                ).reshape(ND, 128, 1),
        ln1_g=np.asarray(inputs["ln1_g"], np.float32).reshape(ND, 128, 1),
        ln1_b=np.asarray(inputs["ln1_b"], np.float32).reshape(ND, 128, 1),
        ln2_g=np.asarray(inputs["ln2_g"], np.float32).reshape(ND, 128, 1),
        ln2_b=np.asarray(inputs["ln2_b"], np.float32).reshape(ND, 128, 1),
    )
    in_maps = []
    for b in range(B):
        xTb = np.ascontiguousarray(x[b].T)  # [D, S]
        x8 = (xTb * 8.0).astype(f8)
        xsw = np.empty((128, NT, 256), f8)
        x8r = x8.reshape(2, 128, S)
        for t0 in range(NT):
            xsw[:, t0, :] = _swi(x8r[:, :, t0 * 128:(t0 + 1) * 128])
        m = dict(shared)
        m["xT"] = xTb.reshape(ND, 128, S)
        m["xD"] = _pack2(x8)
        m["xRD"] = _pack2(_rev_blocks(xTb * 8.0).astype(f8))
        m["xSW"] = xsw
        in_maps.append(m)
    return in_maps


def run_sharded(inputs, trace=False, trace_kwargs=None):
    if "nc" not in _CACHE:
        _CACHE["nc"] = _build()
    nc = _CACHE["nc"]
    in_maps = _prep_inputs(inputs)
    res = run_bass_kernel_spmd(nc, in_maps, list(range(N_CORES)), trace=trace,
                               **(trace_kwargs or {}))
    outs = []
    for b in range(B):
        r = np.asarray(res.results[b]["out"]).astype(np.float32).reshape(D, S)
        outs.append(r.T)
    return np.stack(outs), res


def kernel(**inputs) -> np.ndarray:
    mask = np.asarray(inputs["attention_mask"])
    if not np.all(mask != 0):
        return _np_reference(**{k: np.asarray(v) for k, v in inputs.items()})
    out, _ = run_sharded(inputs, trace=False)
    return out
